# revision 36
# baseline (speedup 1.0000x reference)
"""Trainium2 Bass kernel for nn_CritiGraph (retrieval_knn).

Math: for each (b,i,c,d) the loss is
    total_loss[b,i,c,d] = (1/lg[b,i]) * sum_j mask[b,i,j] *
        | (dist(sta[b,j,d], cnc[b,i,c,d]) - dsp[b,i,j,d] + dsps[b,i,j])/8 - logits[b,i,j] |
with dist(u,w) = sg * (1 - lut[|u| ^ |w|]), sg = +-1 by sign agreement.

The reference lut is (floor(log2(x+1))+1)/16, which equals bitlength(x)/16
for all but ~17 of the 65536 entries (x=0, x=2^m-1, plus XLA float32-log2
quirks at 2^13/2^15).  The device computes bitlength(x)/16 arithmetically
from the float32 exponent of x (convert to f32, shift bits >> 23); the few
exception entries are fixed up exactly by a host-built sparse additive
correction ("delta") that is accumulated into PSUM with one extra matmul.

Folding the signs inside the absolute value gives, per (j, element):
    128*mask*|delt| = | v*mask + bias |,   v = (exp(f32(x)) - 142) * sw,
    bias[i,j,d] = -mask[i,j] * 128*off[i,j,d]*su[j,d]
so the j-loop is: DVE uint16 xor (station tiles DMA-streamed) ->
uint16->f32 copy (split ACT/GPSIMD) -> DVE shift -> DVE (er-142)*sw ->
ACT Abs(v*scale+bias) -> float16 -> PE identity-matmul accumulate into
PSUM (f32).  The raw per-candidate loss accumulator is DMA'd out; the
host takes the argmin, exactly recomputing (reference-faithful, f32)
only candidates within eps of each row minimum so near-ties resolve
identically to jnp.argmin (device acc error bound measured ~0.4).

Sharding: data-parallel over B=8 batch rows -> 8 NeuronCores, one row per
core.  Host prepares per-core transposed views + small (i,j,d) tables; a
pure-numpy fallback handles inputs whose lut is not bitlength-like.
"""

import sys

import numpy as np

sys.path.insert(0, "/opt/trn_rl_repo")

H = 16
NLUT = 1 << H
TP = 8
B = 8
T = 32
C = 513
CA, CB = 257, 256  # PSUM bank split of the candidate axis

_compiled = {}


def _er3_dev_table():
    """Device er3 value per x: (bits(bf16(x)) >> 7) - 142 (u16->bf16 cast)."""
    import ml_dtypes

    bits = np.arange(NLUT, dtype=np.float64).astype(ml_dtypes.bfloat16)
    return (bits.view(np.uint16).astype(np.int64) >> 7) - 142


def _lut_exceptions(lut):
    """Indices where the device bf16-exponent formula disagrees with the lut."""
    want = lut.astype(np.float64) * 16.0 - 16.0
    return np.nonzero(want != _er3_dev_table().astype(np.float64))[0]


def _emit(ctx, tc, outs, ins):
    from concourse import mybir

    nc = tc.nc
    f32 = mybir.dt.float32
    i32 = mybir.dt.int32
    i16 = mybir.dt.int16
    u16 = mybir.dt.uint16
    X = mybir.AxisListType.X
    op = mybir.AluOpType
    AF = mybir.ActivationFunctionType

    bf16 = mybir.dt.bfloat16
    u16 = mybir.dt.uint16
    cnc, auf, btab, mtab, ident, delta = (
        ins["cnc"],
        ins["auf"],
        ins["btab"],
        ins["mtab"],
        ins["ident"],
        ins["delta"],
    )
    out = outs["out"]

    consts = ctx.enter_context(tc.tile_pool(name="consts", bufs=1))
    data = ctx.enter_context(tc.tile_pool(name="data", bufs=1))
    aupool = ctx.enter_context(tc.tile_pool(name="aupool", bufs=6))
    work = ctx.enter_context(tc.tile_pool(name="work", bufs=4))
    accp = ctx.enter_context(tc.tile_pool(name="acc", bufs=1, space="PSUM"))

    btab_sb = consts.tile([128, 2 * T], f32, tag="btab")
    mtab_sb = consts.tile([128, 2 * T], f32, tag="mtab")
    for h in range(2):
        nc.sync.dma_start(btab_sb[:, h * T : (h + 1) * T], btab[h])
        nc.sync.dma_start(mtab_sb[:, h * T : (h + 1) * T], mtab[h])
    f16 = mybir.dt.float16
    ident_sb = consts.tile([128, 128], f16, tag="ident")
    nc.sync.dma_start(ident_sb[:], ident[:])

    for h in range(2):
        cnc_sb = data.tile([128, C], i32, tag=f"cnc{h}")
        nc.sync.dma_start(cnc_sb[:], cnc[h])
        delta_sb = data.tile([128, C], f16, tag=f"delta{h}")
        nc.sync.dma_start(delta_sb[:], delta[h])
        g = work.tile([128, C], i32, tag="g")
        nc.vector.tensor_scalar(g[:], cnc_sb[:], 0, None, op.is_ge)
        swc = data.tile([128, C], i32, tag=f"swc{h}")
        nc.vector.tensor_scalar(swc[:], g[:], 2, -1, op.mult, op.add)
        swc16 = data.tile([128, C], i16, tag=f"swc16{h}")
        nc.vector.tensor_scalar(swc16[:], g[:], 2, -1, op.mult, op.add)
        aw = data.tile([128, C], u16, tag=f"aw{h}")
        nc.vector.tensor_tensor(aw[:], cnc_sb[:], swc[:], op.mult)

        pa = accp.tile([128, CA], f32, tag=f"pa{h}")
        pb = accp.tile([128, CB], f32, tag=f"pb{h}")
        for j in range(T):
            au_t = aupool.tile([128, C], u16, tag="au")
            nc.sync.dma_start(au_t[:], auf[j])
            x16 = work.tile([128, C], u16, tag="x16")
            nc.vector.tensor_tensor(x16[:], aw[:], au_t[:], op.bitwise_xor)
            y = work.tile([128, C], u16, tag="y")
            if j % 2 == 0:
                nc.gpsimd.tensor_copy(y[:].bitcast(bf16), x16[:])
            else:
                nc.scalar.copy(y[:].bitcast(bf16), x16[:])
            er = work.tile([128, C], u16, tag="er")
            nc.vector.tensor_scalar(
                er[:], y[:], 7, None, op.logical_shift_right
            )
            t = work.tile([128, C], f16, tag="t")
            nc.vector.scalar_tensor_tensor(
                t[:], er[:], 142, swc16[:], op.subtract, op.mult
            )
            w = work.tile([128, C], f16, tag="w")
            jj = h * T + j
            nc.scalar.activation(
                w[:],
                t[:],
                AF.Abs,
                bias=btab_sb[:, jj : jj + 1],
                scale=mtab_sb[:, jj : jj + 1],
            )
            nc.tensor.matmul(
                pa[:], ident_sb[:], w[:, 0:CA], start=(j == 0), stop=False
            )
            nc.tensor.matmul(
                pb[:], ident_sb[:], w[:, CA:C], start=(j == 0), stop=False
            )
        nc.tensor.matmul(
            pa[:], ident_sb[:], delta_sb[:, 0:CA], start=False, stop=True
        )
        nc.tensor.matmul(
            pb[:], ident_sb[:], delta_sb[:, CA:C], start=False, stop=True
        )
        acc_sb = data.tile([128, C], f32, tag=f"accsb{h}")
        nc.scalar.copy(acc_sb[:, 0:CA], pa[:])
        nc.scalar.copy(acc_sb[:, CA:C], pb[:])
        nc.sync.dma_start(out[h], acc_sb[:])


def _build():
    if "nc" in _compiled:
        return _compiled["nc"]
    from contextlib import ExitStack

    from concourse import bacc, mybir, tile

    nc = bacc.Bacc("TRN2", target_bir_lowering=False, debug=False)
    f32, i32 = mybir.dt.float32, mybir.dt.int32
    u16, f16 = mybir.dt.uint16, mybir.dt.float16
    ins = {
        "cnc": nc.dram_tensor("cnc", [2, 128, C], i32, kind="ExternalInput").ap(),
        "auf": nc.dram_tensor("auf", [T, 128, C], u16, kind="ExternalInput").ap(),
        "btab": nc.dram_tensor("btab", [2, 128, T], f32, kind="ExternalInput").ap(),
        "mtab": nc.dram_tensor("mtab", [2, 128, T], f32, kind="ExternalInput").ap(),
        "ident": nc.dram_tensor(
            "ident", [128, 128], f16, kind="ExternalInput"
        ).ap(),
        "delta": nc.dram_tensor(
            "delta", [2, 128, C], f16, kind="ExternalInput"
        ).ap(),
    }
    outs = {
        "out": nc.dram_tensor("out", [2, 128, C], f32, kind="ExternalOutput").ap()
    }
    with tile.TileContext(nc) as tc:
        with ExitStack() as ctx:
            _emit(ctx, tc, outs, ins)
    nc.compile()
    _compiled["nc"] = nc
    return nc


def _prep_core(b, sta, cnc, logits, lg, mask, lut, exc):
    abs_s = np.abs(sta[b]).astype(np.int64)  # (T, TP)
    su = np.where(sta[b] >= 0, np.float32(1.0), np.float32(-1.0))
    dx = abs_s[:, None, :] ^ abs_s[None, :, :]
    dsp = (su[:, None, :] * su[None, :, :] * (np.float32(1.0) - lut[dx])).astype(
        np.float32
    )  # (i,j,d)
    dsps = dsp.sum(-1, dtype=np.float32)  # (i,j)
    off = (
        (dsps[:, :, None] - dsp) / np.float32(8.0) - logits[b][:, :, None]
    ).astype(np.float32)
    A = (np.float32(128.0) * off * su[None, :, :]).astype(np.float32)  # (i,j,d)
    m = mask[b].astype(np.float32)  # (i,j)
    bt = -(m[:, :, None] * A)  # (i,j,d)
    bt = np.ascontiguousarray(np.transpose(bt, (0, 2, 1))).reshape(2, 128, T)
    mt = np.ascontiguousarray(
        np.broadcast_to(m[:, None, :], (T, TP, T))
    ).reshape(2, 128, T)
    # au[p=(i,d), j] = abs_s[j, d]; replicated over the 16 i-rows of a half
    au_tab = np.ascontiguousarray(
        np.broadcast_to(abs_s.T[None, :, :], (16, TP, T))
    ).reshape(128, T)
    cnc_t = np.ascontiguousarray(np.transpose(cnc[b], (0, 2, 1))).reshape(
        2, 128, C
    )

    # sparse exception fixup, accumulated into PSUM by the delta matmul
    delta = np.zeros((2, 128, C), dtype=np.float64)
    if len(exc):
        aw = np.abs(cnc_t).astype(np.int64)
        swc = np.where(cnc_t >= 0, 1.0, -1.0)
        er3_true = lut.astype(np.float64) * 16.0 - 16.0
        er3_dev = _er3_dev_table().astype(np.float64)
        is_exc = np.zeros(NLUT, dtype=bool)
        is_exc[exc] = True
        # full xor table (h,p,c,j); patch cells hitting exception x-values
        xs = aw[:, :, :, None] ^ au_tab[None, :, None, :].astype(np.int64)
        hh, pp, cc, jj = np.nonzero(is_exc[xs])
        e = xs[hh, pp, cc, jj]
        sc = mt[hh, pp, jj].astype(np.float64)
        bias = bt[hh, pp, jj].astype(np.float64)
        sw = swc[hh, pp, cc]
        dw = np.abs(er3_true[e] * sw * sc + bias) - np.abs(
            er3_dev[e] * sw * sc + bias
        )
        np.add.at(delta, (hh, pp, cc), dw)
    return {
        "cnc": cnc_t.astype(np.int32),
        "auf": np.ascontiguousarray(
            np.broadcast_to(au_tab.T[:, :, None], (T, 128, C))
        ).astype(np.uint16),
        "btab": bt.astype(np.float32),
        "mtab": mt.astype(np.float32),
        "ident": np.eye(128, dtype=np.float16),
        "delta": delta.astype(np.float16),
    }


def _numpy_ref(sta, cnc, logits, lg, mask, lut):
    sel = np.zeros(sta.shape[:2] + (sta.shape[2],), dtype=cnc.dtype)
    nb, nt = sta.shape[:2]
    D = sta.shape[-1]
    ml = np.zeros_like(sel, dtype=np.float32)

    def dist(c1, c2):
        sg = np.where((c1 >= 0) == (c2 >= 0), np.float32(1.0), np.float32(-1.0))
        s = lut[np.bitwise_xor(np.abs(c1), np.abs(c2))]
        return (sg * (np.float32(1.0) - s)).astype(np.float32)

    for b in range(nb):
        dsp = dist(sta[b][:, None, :], sta[b][None, :, :])  # (T,T,D)
        dsps = dsp.sum(-1, dtype=np.float32)
        lct = dist(sta[b][None, :, None, :], cnc[b][:, None, :, :])  # (i,j,c,d)
        lct = (lct - dsp[:, :, None, :] + dsps[:, :, None, None]) / np.float32(
            D
        )
        delt = (lct - logits[b][:, :, None, None]) * mask[b][:, :, None, None]
        total = np.abs(delt).sum(axis=1, dtype=np.float32) / lg[b][
            :, None, None
        ].astype(np.float32)  # (i,c,d)
        idx = np.argmin(total, axis=1)  # (i,d)
        sel[b] = np.take_along_axis(cnc[b], idx[:, None, :], axis=1)[:, 0, :]
        ml[b] = np.take_along_axis(total, idx[:, None, :], axis=1)[:, 0, :]
    return sel, ml


def kernel(**inputs):
    sta = np.asarray(inputs["sta_loc"])
    cnc = np.asarray(inputs["cnc_loc"])
    logits = np.asarray(inputs["logits"], dtype=np.float32)
    lg = np.asarray(inputs["lg"])
    mask = np.asarray(inputs["mask"])
    lut = np.asarray(inputs["lut"], dtype=np.float32)

    exc = None
    if lut.shape == (NLUT,):
        exc = _lut_exceptions(lut)
    if (
        exc is None
        or len(exc) > 2048
        or np.any(lg == 0)
        or sta.shape != (B, T, TP)
        or cnc.shape != (B, T, C, TP)
        or np.any(np.abs(sta) >= NLUT)
        or np.any(np.abs(cnc.astype(np.int64)) >= NLUT)
    ):
        return _numpy_ref(sta, cnc, logits, lg, mask, lut)

    from concourse.bass_utils import run_bass_kernel_spmd

    nc = _build()
    in_maps = [
        _prep_core(b, sta, cnc, logits, lg, mask, lut, exc) for b in range(B)
    ]
    res = run_bass_kernel_spmd(nc, in_maps, core_ids=list(range(B))).results
    accs = [np.asarray(res[b]["out"]) for b in range(B)]
    return _finish(accs, sta, cnc, logits, lg, mask, lut)


def _finish(accs, sta, cnc, logits, lg, mask, lut, eps=4.0):
    """Argmin from the device's raw loss accumulator.  Candidates within
    eps (raw units = 128*lg*loss) of the row minimum are recomputed exactly
    reference-style so near-ties resolve identically to the reference."""
    sel = np.zeros((B, T, TP), dtype=cnc.dtype)
    ml = np.zeros((B, T, TP), dtype=np.float32)
    one = np.float32(1.0)
    for b in range(B):
        acc = accs[b].reshape(T, TP, C)  # (i,d,c): (2,16,8,C) flattens to i=(h,16)
        thr = acc.min(axis=-1) + np.float32(eps)
        sub = np.abs(sta[b])  # (T,TP)
        def dist(c1, c2):
            sg = np.where((c1 >= 0) == (c2 >= 0), one, -one)
            s = lut[np.bitwise_xor(np.abs(c1), np.abs(c2))]
            return (sg * (one - s)).astype(np.float32)

        dsp = dist(sta[b][:, None, :], sta[b][None, :, :])  # (i,j,d)
        dsps = dsp.sum(-1, dtype=np.float32)  # (i,j)
        for i in range(T):
            for d in range(TP):
                cs = np.nonzero(acc[i, d] <= thr[i, d])[0]
                q = cnc[b][i, cs, d]  # candidate values
                lct = dist(sta[b][:, d][:, None], q[None, :])  # (j, nc)
                lct = (
                    lct - dsp[i, :, d][:, None] + dsps[i, :][:, None]
                ) / np.float32(TP)
                delt = (lct - logits[b][i][:, None]) * mask[b][i][:, None]
                tot = np.abs(delt).sum(axis=0, dtype=np.float32) / lg[b][
                    i
                ].astype(np.float32)
                k = int(np.argmin(tot))
                sel[b, i, d] = cnc[b][i, cs[k], d]
                ml[b, i, d] = tot[k]
    return sel, ml


# revision 39
# speedup vs baseline: 1.2106x; 1.2106x over previous
"""Trainium2 Bass kernel for nn_CritiGraph (retrieval_knn).

Math: for each (b,i,c,d) the loss is
    total_loss[b,i,c,d] = (1/lg[b,i]) * sum_j mask[b,i,j] *
        | (dist(sta[b,j,d], cnc[b,i,c,d]) - dsp[b,i,j,d] + dsps[b,i,j])/8 - logits[b,i,j] |
with dist(u,w) = sg * (1 - lut[|u| ^ |w|]), sg = +-1 by sign agreement.

The reference lut is (floor(log2(x+1))+1)/16 ~= bitlength(x)/16.  The
device casts x (uint16) to bfloat16 and reads the exponent from bits
14..7 (shift >> 7); the ~258 entries where that disagrees with the lut
(x=0, 2^m-1, XLA log2 quirks, bf16 round-up-across-powers) are fixed up
exactly by a host-built sparse additive correction ("delta") accumulated
into PSUM with one extra matmul.

Folding the signs inside the absolute value gives, per (j, element):
    128*mask*|delt| = | v*mask + bias |,   v = (exp(f32(x)) - 142) * sw,
    bias[i,j,d] = -mask[i,j] * 128*off[i,j,d]*su[j,d]
so the j-loop is: DVE uint16 xor (station tiles DMA-streamed) ->
uint16->bf16 copy (split ACT/GPSIMD) -> DVE shift >>7 -> DVE (er-142)*sw ->
ACT Abs(v*scale+bias) -> float16 -> PE identity-matmul accumulate into
PSUM (f32).  The raw per-candidate loss accumulator is DMA'd out; the
host takes the argmin, exactly recomputing (reference-faithful, f32)
only candidates within eps of each row minimum so near-ties resolve
identically to jnp.argmin (device acc error bound measured ~0.4).

Sharding: data-parallel over B=8 batch rows -> 8 NeuronCores, one row per
core.  Host prepares per-core transposed views + small (i,j,d) tables; a
pure-numpy fallback handles inputs whose lut is not bitlength-like.
"""

import sys

import numpy as np

sys.path.insert(0, "/opt/trn_rl_repo")

H = 16
NLUT = 1 << H
TP = 8
B = 8
T = 32
C = 513
CA, CB = 257, 256  # PSUM bank split of the candidate axis

_compiled = {}


def _er3_dev_table():
    """Device er3 value per x: (bits(bf16(x)) >> 7) - 142 (u16->bf16 cast)."""
    import ml_dtypes

    bits = np.arange(NLUT, dtype=np.float64).astype(ml_dtypes.bfloat16)
    return (bits.view(np.uint16).astype(np.int64) >> 7) - 142


def _lut_exceptions(lut):
    """Indices where the device bf16-exponent formula disagrees with the lut."""
    want = lut.astype(np.float64) * 16.0 - 16.0
    return np.nonzero(want != _er3_dev_table().astype(np.float64))[0]


def _emit(ctx, tc, outs, ins):
    from concourse import mybir

    nc = tc.nc
    f32 = mybir.dt.float32
    i32 = mybir.dt.int32
    i16 = mybir.dt.int16
    u16 = mybir.dt.uint16
    X = mybir.AxisListType.X
    op = mybir.AluOpType
    AF = mybir.ActivationFunctionType

    bf16 = mybir.dt.bfloat16
    u16 = mybir.dt.uint16
    cnc, auf, btab, mtab, ident, delta = (
        ins["cnc"],
        ins["auf"],
        ins["btab"],
        ins["mtab"],
        ins["ident"],
        ins["delta"],
    )
    out = outs["out"]

    consts = ctx.enter_context(tc.tile_pool(name="consts", bufs=1))
    data = ctx.enter_context(tc.tile_pool(name="data", bufs=1))
    aupool = ctx.enter_context(tc.tile_pool(name="aupool", bufs=6))
    work = ctx.enter_context(tc.tile_pool(name="work", bufs=4))
    accp = ctx.enter_context(tc.tile_pool(name="acc", bufs=1, space="PSUM"))

    btab_sb = consts.tile([128, 2 * T], f32, tag="btab")
    mtab_sb = consts.tile([128, 2 * T], f32, tag="mtab")
    for h in range(2):
        nc.sync.dma_start(btab_sb[:, h * T : (h + 1) * T], btab[h])
        nc.sync.dma_start(mtab_sb[:, h * T : (h + 1) * T], mtab[h])
    f16 = mybir.dt.float16
    ident_sb = consts.tile([128, 128], f16, tag="ident")
    nc.sync.dma_start(ident_sb[:], ident[:])

    C2 = 2 * C
    cnc_sb = data.tile([128, C2], i32, tag="cnc")
    nc.sync.dma_start(cnc_sb[:], cnc[:])
    delta_sb = data.tile([128, C2], f16, tag="delta")
    nc.sync.dma_start(delta_sb[:], delta[:])
    g = work.tile([128, C2], i32, tag="g")
    nc.vector.tensor_scalar(g[:], cnc_sb[:], 0, None, op.is_ge)
    swc = data.tile([128, C2], i32, tag="swc")
    nc.vector.tensor_scalar(swc[:], g[:], 2, -1, op.mult, op.add)
    swc16 = data.tile([128, C2], i16, tag="swc16")
    nc.vector.tensor_scalar(swc16[:], g[:], 2, -1, op.mult, op.add)
    aw = data.tile([128, C2], u16, tag="aw")
    nc.vector.tensor_tensor(aw[:], cnc_sb[:], swc[:], op.mult)

    pp = [
        accp.tile([128, CA], f32, tag="pa0", name="pa0"),
        accp.tile([128, CB], f32, tag="pb0", name="pb0"),
        accp.tile([128, CA], f32, tag="pa1", name="pa1"),
        accp.tile([128, CB], f32, tag="pb1", name="pb1"),
    ]
    SL = [(0, CA), (CA, C), (C, C + CA), (C + CA, C2)]
    for j in range(T):
        au_t = aupool.tile([128, C2], u16, tag="au")
        nc.sync.dma_start(au_t[:], auf[j])
        x16 = work.tile([128, C2], u16, tag="x16")
        nc.vector.tensor_tensor(x16[:], aw[:], au_t[:], op.bitwise_xor)
        y = work.tile([128, C2], u16, tag="y")
        if j % 2 == 0:
            nc.gpsimd.tensor_copy(y[:].bitcast(bf16), x16[:])
        else:
            nc.scalar.copy(y[:].bitcast(bf16), x16[:])
        er = work.tile([128, C2], u16, tag="er")
        nc.vector.tensor_scalar(er[:], y[:], 7, None, op.logical_shift_right)
        t = work.tile([128, C2], f16, tag="t")
        nc.vector.scalar_tensor_tensor(
            t[:], er[:], 142, swc16[:], op.subtract, op.mult
        )
        w = work.tile([128, C2], f16, tag="w")
        for h in range(2):
            jj = h * T + j
            nc.scalar.activation(
                w[:, h * C : (h + 1) * C],
                t[:, h * C : (h + 1) * C],
                AF.Abs,
                bias=btab_sb[:, jj : jj + 1],
                scale=mtab_sb[:, jj : jj + 1],
            )
        for k, (a, bnd) in enumerate(SL):
            nc.tensor.matmul(
                pp[k][:], ident_sb[:], w[:, a:bnd], start=(j == 0), stop=False
            )
    for k, (a, bnd) in enumerate(SL):
        nc.tensor.matmul(
            pp[k][:], ident_sb[:], delta_sb[:, a:bnd], start=False, stop=True
        )
    acc_sb = data.tile([128, C2], f32, tag="accsb")
    for k, (a, bnd) in enumerate(SL):
        nc.scalar.copy(acc_sb[:, a:bnd], pp[k][:])
    nc.sync.dma_start(out[0], acc_sb[:, 0:C])
    nc.sync.dma_start(out[1], acc_sb[:, C:C2])


def _build():
    if "nc" in _compiled:
        return _compiled["nc"]
    from contextlib import ExitStack

    from concourse import bacc, mybir, tile

    nc = bacc.Bacc("TRN2", target_bir_lowering=False, debug=False)
    f32, i32 = mybir.dt.float32, mybir.dt.int32
    u16, f16 = mybir.dt.uint16, mybir.dt.float16
    ins = {
        "cnc": nc.dram_tensor("cnc", [128, 2 * C], i32, kind="ExternalInput").ap(),
        "auf": nc.dram_tensor("auf", [T, 128, 2 * C], u16, kind="ExternalInput").ap(),
        "btab": nc.dram_tensor("btab", [2, 128, T], f32, kind="ExternalInput").ap(),
        "mtab": nc.dram_tensor("mtab", [2, 128, T], f32, kind="ExternalInput").ap(),
        "ident": nc.dram_tensor(
            "ident", [128, 128], f16, kind="ExternalInput"
        ).ap(),
        "delta": nc.dram_tensor(
            "delta", [128, 2 * C], f16, kind="ExternalInput"
        ).ap(),
    }
    outs = {
        "out": nc.dram_tensor("out", [2, 128, C], f32, kind="ExternalOutput").ap()
    }
    with tile.TileContext(nc) as tc:
        with ExitStack() as ctx:
            _emit(ctx, tc, outs, ins)
    nc.compile()
    _compiled["nc"] = nc
    return nc


def _prep_core(b, sta, cnc, logits, lg, mask, lut, exc):
    abs_s = np.abs(sta[b]).astype(np.int64)  # (T, TP)
    su = np.where(sta[b] >= 0, np.float32(1.0), np.float32(-1.0))
    dx = abs_s[:, None, :] ^ abs_s[None, :, :]
    dsp = (su[:, None, :] * su[None, :, :] * (np.float32(1.0) - lut[dx])).astype(
        np.float32
    )  # (i,j,d)
    dsps = dsp.sum(-1, dtype=np.float32)  # (i,j)
    off = (
        (dsps[:, :, None] - dsp) / np.float32(8.0) - logits[b][:, :, None]
    ).astype(np.float32)
    A = (np.float32(128.0) * off * su[None, :, :]).astype(np.float32)  # (i,j,d)
    m = mask[b].astype(np.float32)  # (i,j)
    bt = -(m[:, :, None] * A)  # (i,j,d)
    bt = np.ascontiguousarray(np.transpose(bt, (0, 2, 1))).reshape(2, 128, T)
    mt = np.ascontiguousarray(
        np.broadcast_to(m[:, None, :], (T, TP, T))
    ).reshape(2, 128, T)
    # au[p=(i,d), j] = abs_s[j, d]; replicated over the 16 i-rows of a half
    au_tab = np.ascontiguousarray(
        np.broadcast_to(abs_s.T[None, :, :], (16, TP, T))
    ).reshape(128, T)
    cnc_t = np.ascontiguousarray(np.transpose(cnc[b], (0, 2, 1))).reshape(
        2, 128, C
    )

    # sparse exception fixup, accumulated into PSUM by the delta matmul
    delta = np.zeros((2, 128, C), dtype=np.float64)
    if len(exc):
        aw = np.abs(cnc_t).astype(np.int64)
        swc = np.where(cnc_t >= 0, 1.0, -1.0)
        er3_true = lut.astype(np.float64) * 16.0 - 16.0
        er3_dev = _er3_dev_table().astype(np.float64)
        is_exc = np.zeros(NLUT, dtype=bool)
        is_exc[exc] = True
        # full xor table (h,p,c,j); patch cells hitting exception x-values
        xs = aw[:, :, :, None] ^ au_tab[None, :, None, :].astype(np.int64)
        hh, pp, cc, jj = np.nonzero(is_exc[xs])
        e = xs[hh, pp, cc, jj]
        sc = mt[hh, pp, jj].astype(np.float64)
        bias = bt[hh, pp, jj].astype(np.float64)
        sw = swc[hh, pp, cc]
        dw = np.abs(er3_true[e] * sw * sc + bias) - np.abs(
            er3_dev[e] * sw * sc + bias
        )
        np.add.at(delta, (hh, pp, cc), dw)
    return {
        "cnc": np.ascontiguousarray(
            np.transpose(cnc_t, (1, 0, 2))
        ).reshape(128, 2 * C).astype(np.int32),
        "auf": np.ascontiguousarray(
            np.broadcast_to(au_tab.T[:, :, None], (T, 128, 2 * C))
        ).astype(np.uint16),
        "btab": bt.astype(np.float32),
        "mtab": mt.astype(np.float32),
        "ident": np.eye(128, dtype=np.float16),
        "delta": np.ascontiguousarray(
            np.transpose(delta, (1, 0, 2))
        ).reshape(128, 2 * C).astype(np.float16),
    }


def _numpy_ref(sta, cnc, logits, lg, mask, lut):
    sel = np.zeros(sta.shape[:2] + (sta.shape[2],), dtype=cnc.dtype)
    nb, nt = sta.shape[:2]
    D = sta.shape[-1]
    ml = np.zeros_like(sel, dtype=np.float32)

    def dist(c1, c2):
        sg = np.where((c1 >= 0) == (c2 >= 0), np.float32(1.0), np.float32(-1.0))
        s = lut[np.bitwise_xor(np.abs(c1), np.abs(c2))]
        return (sg * (np.float32(1.0) - s)).astype(np.float32)

    for b in range(nb):
        dsp = dist(sta[b][:, None, :], sta[b][None, :, :])  # (T,T,D)
        dsps = dsp.sum(-1, dtype=np.float32)
        lct = dist(sta[b][None, :, None, :], cnc[b][:, None, :, :])  # (i,j,c,d)
        lct = (lct - dsp[:, :, None, :] + dsps[:, :, None, None]) / np.float32(
            D
        )
        delt = (lct - logits[b][:, :, None, None]) * mask[b][:, :, None, None]
        total = np.abs(delt).sum(axis=1, dtype=np.float32) / lg[b][
            :, None, None
        ].astype(np.float32)  # (i,c,d)
        idx = np.argmin(total, axis=1)  # (i,d)
        sel[b] = np.take_along_axis(cnc[b], idx[:, None, :], axis=1)[:, 0, :]
        ml[b] = np.take_along_axis(total, idx[:, None, :], axis=1)[:, 0, :]
    return sel, ml


def kernel(**inputs):
    sta = np.asarray(inputs["sta_loc"])
    cnc = np.asarray(inputs["cnc_loc"])
    logits = np.asarray(inputs["logits"], dtype=np.float32)
    lg = np.asarray(inputs["lg"])
    mask = np.asarray(inputs["mask"])
    lut = np.asarray(inputs["lut"], dtype=np.float32)

    exc = None
    if lut.shape == (NLUT,):
        exc = _lut_exceptions(lut)
    if (
        exc is None
        or len(exc) > 2048
        or np.any(lg == 0)
        or sta.shape != (B, T, TP)
        or cnc.shape != (B, T, C, TP)
        or np.any(np.abs(sta) >= NLUT)
        or np.any(np.abs(cnc.astype(np.int64)) >= NLUT)
    ):
        return _numpy_ref(sta, cnc, logits, lg, mask, lut)

    from concourse.bass_utils import run_bass_kernel_spmd

    nc = _build()
    in_maps = [
        _prep_core(b, sta, cnc, logits, lg, mask, lut, exc) for b in range(B)
    ]
    res = run_bass_kernel_spmd(nc, in_maps, core_ids=list(range(B))).results
    accs = [np.asarray(res[b]["out"]) for b in range(B)]
    return _finish(accs, sta, cnc, logits, lg, mask, lut)


def _finish(accs, sta, cnc, logits, lg, mask, lut, eps=4.0):
    """Argmin from the device's raw loss accumulator.  Candidates within
    eps (raw units = 128*lg*loss) of the row minimum are recomputed exactly
    reference-style so near-ties resolve identically to the reference."""
    sel = np.zeros((B, T, TP), dtype=cnc.dtype)
    ml = np.zeros((B, T, TP), dtype=np.float32)
    one = np.float32(1.0)
    for b in range(B):
        acc = accs[b].reshape(T, TP, C)  # (i,d,c): (2,16,8,C) flattens to i=(h,16)
        thr = acc.min(axis=-1) + np.float32(eps)
        sub = np.abs(sta[b])  # (T,TP)
        def dist(c1, c2):
            sg = np.where((c1 >= 0) == (c2 >= 0), one, -one)
            s = lut[np.bitwise_xor(np.abs(c1), np.abs(c2))]
            return (sg * (one - s)).astype(np.float32)

        dsp = dist(sta[b][:, None, :], sta[b][None, :, :])  # (i,j,d)
        dsps = dsp.sum(-1, dtype=np.float32)  # (i,j)
        for i in range(T):
            for d in range(TP):
                cs = np.nonzero(acc[i, d] <= thr[i, d])[0]
                q = cnc[b][i, cs, d]  # candidate values
                lct = dist(sta[b][:, d][:, None], q[None, :])  # (j, nc)
                lct = (
                    lct - dsp[i, :, d][:, None] + dsps[i, :][:, None]
                ) / np.float32(TP)
                delt = (lct - logits[b][i][:, None]) * mask[b][i][:, None]
                tot = np.abs(delt).sum(axis=0, dtype=np.float32) / lg[b][
                    i
                ].astype(np.float32)
                k = int(np.argmin(tot))
                sel[b, i, d] = cnc[b][i, cs[k], d]
                ml[b, i, d] = tot[k]
    return sel, ml


# revision 40
# speedup vs baseline: 1.2132x; 1.0021x over previous
"""Trainium2 Bass kernel for nn_CritiGraph (retrieval_knn).

Math: for each (b,i,c,d) the loss is
    total_loss[b,i,c,d] = (1/lg[b,i]) * sum_j mask[b,i,j] *
        | (dist(sta[b,j,d], cnc[b,i,c,d]) - dsp[b,i,j,d] + dsps[b,i,j])/8 - logits[b,i,j] |
with dist(u,w) = sg * (1 - lut[|u| ^ |w|]), sg = +-1 by sign agreement.

The reference lut is (floor(log2(x+1))+1)/16 ~= bitlength(x)/16.  The
device casts x (uint16) to bfloat16 and reads the exponent from bits
14..7 (shift >> 7); the ~258 entries where that disagrees with the lut
(x=0, 2^m-1, XLA log2 quirks, bf16 round-up-across-powers) are fixed up
exactly by a host-built sparse additive correction ("delta") accumulated
into PSUM with one extra matmul.

Folding the signs inside the absolute value gives, per (j, element):
    128*mask*|delt| = | v*mask + bias |,   v = (exp(f32(x)) - 142) * sw,
    bias[i,j,d] = -mask[i,j] * 128*off[i,j,d]*su[j,d]
so the j-loop is: DVE uint16 xor (station tiles DMA-streamed) ->
uint16->bf16 copy (split ACT/GPSIMD) -> DVE shift >>7 -> DVE (er-142)*sw ->
ACT Abs(v*scale+bias) -> float16 -> PE identity-matmul accumulate into
PSUM (f32).  The raw per-candidate loss accumulator is DMA'd out; the
host takes the argmin, exactly recomputing (reference-faithful, f32)
only candidates within eps of each row minimum so near-ties resolve
identically to jnp.argmin (device acc error bound measured ~0.4).

Sharding: data-parallel over B=8 batch rows -> 8 NeuronCores, one row per
core.  Host prepares per-core transposed views + small (i,j,d) tables; a
pure-numpy fallback handles inputs whose lut is not bitlength-like.
"""

import sys

import numpy as np

sys.path.insert(0, "/opt/trn_rl_repo")

H = 16
NLUT = 1 << H
TP = 8
B = 8
T = 32
C = 513
CA, CB = 257, 256  # PSUM bank split of the candidate axis

_compiled = {}


def _er3_dev_table():
    """Device er3 value per x: (bits(bf16(x)) >> 7) - 142 (u16->bf16 cast)."""
    import ml_dtypes

    bits = np.arange(NLUT, dtype=np.float64).astype(ml_dtypes.bfloat16)
    return (bits.view(np.uint16).astype(np.int64) >> 7) - 142


def _lut_exceptions(lut):
    """Indices where the device bf16-exponent formula disagrees with the lut."""
    want = lut.astype(np.float64) * 16.0 - 16.0
    return np.nonzero(want != _er3_dev_table().astype(np.float64))[0]


def _emit(ctx, tc, outs, ins):
    from concourse import mybir

    nc = tc.nc
    f32 = mybir.dt.float32
    i32 = mybir.dt.int32
    i16 = mybir.dt.int16
    u16 = mybir.dt.uint16
    X = mybir.AxisListType.X
    op = mybir.AluOpType
    AF = mybir.ActivationFunctionType

    bf16 = mybir.dt.bfloat16
    u16 = mybir.dt.uint16
    cnc, auf, btab, mtab, ident, delta = (
        ins["cnc"],
        ins["auf"],
        ins["btab"],
        ins["mtab"],
        ins["ident"],
        ins["delta"],
    )
    out = outs["out"]

    consts = ctx.enter_context(tc.tile_pool(name="consts", bufs=1))
    data = ctx.enter_context(tc.tile_pool(name="data", bufs=1))
    aupool = ctx.enter_context(tc.tile_pool(name="aupool", bufs=8))
    work = ctx.enter_context(tc.tile_pool(name="work", bufs=6))
    accp = ctx.enter_context(tc.tile_pool(name="acc", bufs=1, space="PSUM"))

    btab_sb = consts.tile([128, 2 * T], f32, tag="btab")
    mtab_sb = consts.tile([128, 2 * T], f32, tag="mtab")
    for h in range(2):
        nc.sync.dma_start(btab_sb[:, h * T : (h + 1) * T], btab[h])
        nc.sync.dma_start(mtab_sb[:, h * T : (h + 1) * T], mtab[h])
    f16 = mybir.dt.float16
    ident_sb = consts.tile([128, 128], f16, tag="ident")
    nc.sync.dma_start(ident_sb[:], ident[:])

    C2 = 2 * C
    cnc_sb = data.tile([128, C2], i32, tag="cnc")
    nc.sync.dma_start(cnc_sb[:], cnc[:])
    delta_sb = data.tile([128, C2], f16, tag="delta")
    nc.sync.dma_start(delta_sb[:], delta[:])
    g = work.tile([128, C2], i32, tag="g")
    nc.vector.tensor_scalar(g[:], cnc_sb[:], 0, None, op.is_ge)
    swc = data.tile([128, C2], i32, tag="swc")
    nc.vector.tensor_scalar(swc[:], g[:], 2, -1, op.mult, op.add)
    swc16 = data.tile([128, C2], i16, tag="swc16")
    nc.vector.tensor_scalar(swc16[:], g[:], 2, -1, op.mult, op.add)
    aw = data.tile([128, C2], u16, tag="aw")
    nc.vector.tensor_tensor(aw[:], cnc_sb[:], swc[:], op.mult)

    pp = [
        accp.tile([128, CA], f32, tag="pa0", name="pa0"),
        accp.tile([128, CB], f32, tag="pb0", name="pb0"),
        accp.tile([128, CA], f32, tag="pa1", name="pa1"),
        accp.tile([128, CB], f32, tag="pb1", name="pb1"),
    ]
    SL = [(0, CA), (CA, C), (C, C + CA), (C + CA, C2)]
    for j in range(T):
        au_t = aupool.tile([128, C2], u16, tag="au")
        nc.sync.dma_start(au_t[:], auf[j])
        x16 = work.tile([128, C2], u16, tag="x16")
        nc.vector.tensor_tensor(x16[:], aw[:], au_t[:], op.bitwise_xor)
        y = work.tile([128, C2], u16, tag="y")
        if j % 2 == 0:
            nc.gpsimd.tensor_copy(y[:].bitcast(bf16), x16[:])
        else:
            nc.scalar.copy(y[:].bitcast(bf16), x16[:])
        er = work.tile([128, C2], u16, tag="er")
        nc.vector.tensor_scalar(er[:], y[:], 7, None, op.logical_shift_right)
        t = work.tile([128, C2], f16, tag="t")
        nc.vector.scalar_tensor_tensor(
            t[:], er[:], 142, swc16[:], op.subtract, op.mult
        )
        w = work.tile([128, C2], f16, tag="w")
        for h in range(2):
            jj = h * T + j
            nc.scalar.activation(
                w[:, h * C : (h + 1) * C],
                t[:, h * C : (h + 1) * C],
                AF.Abs,
                bias=btab_sb[:, jj : jj + 1],
                scale=mtab_sb[:, jj : jj + 1],
            )
        for k, (a, bnd) in enumerate(SL):
            nc.tensor.matmul(
                pp[k][:], ident_sb[:], w[:, a:bnd], start=(j == 0), stop=False
            )
    for k, (a, bnd) in enumerate(SL):
        nc.tensor.matmul(
            pp[k][:], ident_sb[:], delta_sb[:, a:bnd], start=False, stop=True
        )
    acc_sb = data.tile([128, C2], f32, tag="accsb")
    for k, (a, bnd) in enumerate(SL):
        nc.scalar.copy(acc_sb[:, a:bnd], pp[k][:])
    nc.sync.dma_start(out[0], acc_sb[:, 0:C])
    nc.sync.dma_start(out[1], acc_sb[:, C:C2])


def _build():
    if "nc" in _compiled:
        return _compiled["nc"]
    from contextlib import ExitStack

    from concourse import bacc, mybir, tile

    nc = bacc.Bacc("TRN2", target_bir_lowering=False, debug=False)
    f32, i32 = mybir.dt.float32, mybir.dt.int32
    u16, f16 = mybir.dt.uint16, mybir.dt.float16
    ins = {
        "cnc": nc.dram_tensor("cnc", [128, 2 * C], i32, kind="ExternalInput").ap(),
        "auf": nc.dram_tensor("auf", [T, 128, 2 * C], u16, kind="ExternalInput").ap(),
        "btab": nc.dram_tensor("btab", [2, 128, T], f32, kind="ExternalInput").ap(),
        "mtab": nc.dram_tensor("mtab", [2, 128, T], f32, kind="ExternalInput").ap(),
        "ident": nc.dram_tensor(
            "ident", [128, 128], f16, kind="ExternalInput"
        ).ap(),
        "delta": nc.dram_tensor(
            "delta", [128, 2 * C], f16, kind="ExternalInput"
        ).ap(),
    }
    outs = {
        "out": nc.dram_tensor("out", [2, 128, C], f32, kind="ExternalOutput").ap()
    }
    with tile.TileContext(nc) as tc:
        with ExitStack() as ctx:
            _emit(ctx, tc, outs, ins)
    nc.compile()
    _compiled["nc"] = nc
    return nc


def _prep_core(b, sta, cnc, logits, lg, mask, lut, exc):
    abs_s = np.abs(sta[b]).astype(np.int64)  # (T, TP)
    su = np.where(sta[b] >= 0, np.float32(1.0), np.float32(-1.0))
    dx = abs_s[:, None, :] ^ abs_s[None, :, :]
    dsp = (su[:, None, :] * su[None, :, :] * (np.float32(1.0) - lut[dx])).astype(
        np.float32
    )  # (i,j,d)
    dsps = dsp.sum(-1, dtype=np.float32)  # (i,j)
    off = (
        (dsps[:, :, None] - dsp) / np.float32(8.0) - logits[b][:, :, None]
    ).astype(np.float32)
    A = (np.float32(128.0) * off * su[None, :, :]).astype(np.float32)  # (i,j,d)
    m = mask[b].astype(np.float32)  # (i,j)
    bt = -(m[:, :, None] * A)  # (i,j,d)
    bt = np.ascontiguousarray(np.transpose(bt, (0, 2, 1))).reshape(2, 128, T)
    mt = np.ascontiguousarray(
        np.broadcast_to(m[:, None, :], (T, TP, T))
    ).reshape(2, 128, T)
    # au[p=(i,d), j] = abs_s[j, d]; replicated over the 16 i-rows of a half
    au_tab = np.ascontiguousarray(
        np.broadcast_to(abs_s.T[None, :, :], (16, TP, T))
    ).reshape(128, T)
    cnc_t = np.ascontiguousarray(np.transpose(cnc[b], (0, 2, 1))).reshape(
        2, 128, C
    )

    # sparse exception fixup, accumulated into PSUM by the delta matmul
    delta = np.zeros((2, 128, C), dtype=np.float64)
    if len(exc):
        aw = np.abs(cnc_t).astype(np.int64)
        swc = np.where(cnc_t >= 0, 1.0, -1.0)
        er3_true = lut.astype(np.float64) * 16.0 - 16.0
        er3_dev = _er3_dev_table().astype(np.float64)
        is_exc = np.zeros(NLUT, dtype=bool)
        is_exc[exc] = True
        # full xor table (h,p,c,j); patch cells hitting exception x-values
        xs = aw[:, :, :, None] ^ au_tab[None, :, None, :].astype(np.int64)
        hh, pp, cc, jj = np.nonzero(is_exc[xs])
        e = xs[hh, pp, cc, jj]
        sc = mt[hh, pp, jj].astype(np.float64)
        bias = bt[hh, pp, jj].astype(np.float64)
        sw = swc[hh, pp, cc]
        dw = np.abs(er3_true[e] * sw * sc + bias) - np.abs(
            er3_dev[e] * sw * sc + bias
        )
        np.add.at(delta, (hh, pp, cc), dw)
    return {
        "cnc": np.ascontiguousarray(
            np.transpose(cnc_t, (1, 0, 2))
        ).reshape(128, 2 * C).astype(np.int32),
        "auf": np.ascontiguousarray(
            np.broadcast_to(au_tab.T[:, :, None], (T, 128, 2 * C))
        ).astype(np.uint16),
        "btab": bt.astype(np.float32),
        "mtab": mt.astype(np.float32),
        "ident": np.eye(128, dtype=np.float16),
        "delta": np.ascontiguousarray(
            np.transpose(delta, (1, 0, 2))
        ).reshape(128, 2 * C).astype(np.float16),
    }


def _numpy_ref(sta, cnc, logits, lg, mask, lut):
    sel = np.zeros(sta.shape[:2] + (sta.shape[2],), dtype=cnc.dtype)
    nb, nt = sta.shape[:2]
    D = sta.shape[-1]
    ml = np.zeros_like(sel, dtype=np.float32)

    def dist(c1, c2):
        sg = np.where((c1 >= 0) == (c2 >= 0), np.float32(1.0), np.float32(-1.0))
        s = lut[np.bitwise_xor(np.abs(c1), np.abs(c2))]
        return (sg * (np.float32(1.0) - s)).astype(np.float32)

    for b in range(nb):
        dsp = dist(sta[b][:, None, :], sta[b][None, :, :])  # (T,T,D)
        dsps = dsp.sum(-1, dtype=np.float32)
        lct = dist(sta[b][None, :, None, :], cnc[b][:, None, :, :])  # (i,j,c,d)
        lct = (lct - dsp[:, :, None, :] + dsps[:, :, None, None]) / np.float32(
            D
        )
        delt = (lct - logits[b][:, :, None, None]) * mask[b][:, :, None, None]
        total = np.abs(delt).sum(axis=1, dtype=np.float32) / lg[b][
            :, None, None
        ].astype(np.float32)  # (i,c,d)
        idx = np.argmin(total, axis=1)  # (i,d)
        sel[b] = np.take_along_axis(cnc[b], idx[:, None, :], axis=1)[:, 0, :]
        ml[b] = np.take_along_axis(total, idx[:, None, :], axis=1)[:, 0, :]
    return sel, ml


def kernel(**inputs):
    sta = np.asarray(inputs["sta_loc"])
    cnc = np.asarray(inputs["cnc_loc"])
    logits = np.asarray(inputs["logits"], dtype=np.float32)
    lg = np.asarray(inputs["lg"])
    mask = np.asarray(inputs["mask"])
    lut = np.asarray(inputs["lut"], dtype=np.float32)

    exc = None
    if lut.shape == (NLUT,):
        exc = _lut_exceptions(lut)
    if (
        exc is None
        or len(exc) > 2048
        or np.any(lg == 0)
        or sta.shape != (B, T, TP)
        or cnc.shape != (B, T, C, TP)
        or np.any(np.abs(sta) >= NLUT)
        or np.any(np.abs(cnc.astype(np.int64)) >= NLUT)
    ):
        return _numpy_ref(sta, cnc, logits, lg, mask, lut)

    from concourse.bass_utils import run_bass_kernel_spmd

    nc = _build()
    in_maps = [
        _prep_core(b, sta, cnc, logits, lg, mask, lut, exc) for b in range(B)
    ]
    res = run_bass_kernel_spmd(nc, in_maps, core_ids=list(range(B))).results
    accs = [np.asarray(res[b]["out"]) for b in range(B)]
    return _finish(accs, sta, cnc, logits, lg, mask, lut)


def _finish(accs, sta, cnc, logits, lg, mask, lut, eps=4.0):
    """Argmin from the device's raw loss accumulator.  Candidates within
    eps (raw units = 128*lg*loss) of the row minimum are recomputed exactly
    reference-style so near-ties resolve identically to the reference."""
    sel = np.zeros((B, T, TP), dtype=cnc.dtype)
    ml = np.zeros((B, T, TP), dtype=np.float32)
    one = np.float32(1.0)
    for b in range(B):
        acc = accs[b].reshape(T, TP, C)  # (i,d,c): (2,16,8,C) flattens to i=(h,16)
        thr = acc.min(axis=-1) + np.float32(eps)
        sub = np.abs(sta[b])  # (T,TP)
        def dist(c1, c2):
            sg = np.where((c1 >= 0) == (c2 >= 0), one, -one)
            s = lut[np.bitwise_xor(np.abs(c1), np.abs(c2))]
            return (sg * (one - s)).astype(np.float32)

        dsp = dist(sta[b][:, None, :], sta[b][None, :, :])  # (i,j,d)
        dsps = dsp.sum(-1, dtype=np.float32)  # (i,j)
        for i in range(T):
            for d in range(TP):
                cs = np.nonzero(acc[i, d] <= thr[i, d])[0]
                q = cnc[b][i, cs, d]  # candidate values
                lct = dist(sta[b][:, d][:, None], q[None, :])  # (j, nc)
                lct = (
                    lct - dsp[i, :, d][:, None] + dsps[i, :][:, None]
                ) / np.float32(TP)
                delt = (lct - logits[b][i][:, None]) * mask[b][i][:, None]
                tot = np.abs(delt).sum(axis=0, dtype=np.float32) / lg[b][
                    i
                ].astype(np.float32)
                k = int(np.argmin(tot))
                sel[b, i, d] = cnc[b][i, cs[k], d]
                ml[b, i, d] = tot[k]
    return sel, ml


# revision 41
# speedup vs baseline: 1.4562x; 1.2003x over previous
"""Trainium2 Bass kernel for nn_CritiGraph (retrieval_knn).

Math: for each (b,i,c,d) the loss is
    total_loss[b,i,c,d] = (1/lg[b,i]) * sum_j mask[b,i,j] *
        | (dist(sta[b,j,d], cnc[b,i,c,d]) - dsp[b,i,j,d] + dsps[b,i,j])/8 - logits[b,i,j] |
with dist(u,w) = sg * (1 - lut[|u| ^ |w|]), sg = +-1 by sign agreement.

The reference lut is (floor(log2(x+1))+1)/16 ~= bitlength(x)/16.  The
device casts x (uint16) to bfloat16 and reads the exponent from bits
14..7 (shift >> 7); the ~258 entries where that disagrees with the lut
(x=0, 2^m-1, XLA log2 quirks, bf16 round-up-across-powers) are fixed up
exactly by a host-built sparse additive correction ("delta") accumulated
into PSUM with one extra matmul.

Folding the signs inside the absolute value gives, per (j, element):
    128*mask*|delt| = | v*mask + bias |,   v = (exp(f32(x)) - 142) * sw,
    bias[i,j,d] = -mask[i,j] * 128*off[i,j,d]*su[j,d]
so the j-loop is: DVE uint16 xor (station tiles DMA-streamed) ->
uint16->bf16 copy (split ACT/GPSIMD) -> DVE shift >>7 -> DVE (er-142)*sw ->
ACT Abs(v*scale+bias) -> float16 -> PE identity-matmul accumulate into
PSUM (f32).  The raw per-candidate loss accumulator is DMA'd out; the
host takes the argmin, exactly recomputing (reference-faithful, f32)
only candidates within eps of each row minimum so near-ties resolve
identically to jnp.argmin (device acc error bound measured ~0.4).

Sharding: data-parallel over B=8 batch rows -> 8 NeuronCores, one row per
core.  Host prepares per-core transposed views + small (i,j,d) tables; a
pure-numpy fallback handles inputs whose lut is not bitlength-like.
"""

import sys

import numpy as np

sys.path.insert(0, "/opt/trn_rl_repo")

H = 16
NLUT = 1 << H
TP = 8
B = 8
T = 32
C = 513
CA, CB = 257, 256  # PSUM bank split of the candidate axis

_compiled = {}


def _er3_dev_table():
    """Device er3 value per x: (bits(bf16(x)) >> 7) - 142 (u16->bf16 cast)."""
    import ml_dtypes

    bits = np.arange(NLUT, dtype=np.float64).astype(ml_dtypes.bfloat16)
    return (bits.view(np.uint16).astype(np.int64) >> 7) - 142


def _lut_exceptions(lut):
    """Indices where the device bf16-exponent formula disagrees with the lut."""
    want = lut.astype(np.float64) * 16.0 - 16.0
    return np.nonzero(want != _er3_dev_table().astype(np.float64))[0]


def _emit(ctx, tc, outs, ins):
    from concourse import mybir

    nc = tc.nc
    f32 = mybir.dt.float32
    i32 = mybir.dt.int32
    i16 = mybir.dt.int16
    u16 = mybir.dt.uint16
    X = mybir.AxisListType.X
    op = mybir.AluOpType
    AF = mybir.ActivationFunctionType

    bf16 = mybir.dt.bfloat16
    u16 = mybir.dt.uint16
    cnc, auf, btab, mtab, ident, delta = (
        ins["cnc"],
        ins["auf"],
        ins["btab"],
        ins["mtab"],
        ins["ident"],
        ins["delta"],
    )
    out = outs["out"]

    consts = ctx.enter_context(tc.tile_pool(name="consts", bufs=1))
    data = ctx.enter_context(tc.tile_pool(name="data", bufs=1))
    aupool = ctx.enter_context(tc.tile_pool(name="aupool", bufs=8))
    work = ctx.enter_context(tc.tile_pool(name="work", bufs=8))
    accp = ctx.enter_context(tc.tile_pool(name="acc", bufs=1, space="PSUM"))

    btab_sb = consts.tile([128, 2 * T], f32, tag="btab")
    mtab_sb = consts.tile([128, 2 * T], f32, tag="mtab")
    for h in range(2):
        nc.sync.dma_start(btab_sb[:, h * T : (h + 1) * T], btab[h])
        nc.sync.dma_start(mtab_sb[:, h * T : (h + 1) * T], mtab[h])
    f16 = mybir.dt.float16
    ident_sb = consts.tile([128, 128], f16, tag="ident")
    nc.sync.dma_start(ident_sb[:], ident[:])

    C2 = 2 * C
    cnc_sb = data.tile([128, C2], i32, tag="cnc")
    nc.sync.dma_start(cnc_sb[:], cnc[:])
    delta_sb = data.tile([128, C2], f16, tag="delta")
    nc.sync.dma_start(delta_sb[:], delta[:])
    g = work.tile([128, C2], i32, tag="g")
    nc.vector.tensor_scalar(g[:], cnc_sb[:], 0, None, op.is_ge)
    swc = data.tile([128, C2], i32, tag="swc")
    nc.vector.tensor_scalar(swc[:], g[:], 2, -1, op.mult, op.add)
    swc16 = data.tile([128, C2], i16, tag="swc16")
    nc.vector.tensor_scalar(swc16[:], g[:], 2, -1, op.mult, op.add)
    aw = data.tile([128, C2], u16, tag="aw")
    nc.vector.tensor_tensor(aw[:], cnc_sb[:], swc[:], op.mult)

    pp = [
        accp.tile([128, CA], f32, tag="pa0", name="pa0"),
        accp.tile([128, CB], f32, tag="pb0", name="pb0"),
        accp.tile([128, CA], f32, tag="pa1", name="pa1"),
        accp.tile([128, CB], f32, tag="pb1", name="pb1"),
    ]
    SL = [(0, CA), (CA, C), (C, C + CA), (C + CA, C2)]
    for j in range(T):
        au_t = aupool.tile([128, C2], u16, tag="au")
        nc.sync.dma_start(au_t[:], auf[j])
        x16 = work.tile([128, C2], u16, tag="x16")
        nc.vector.tensor_tensor(x16[:], aw[:], au_t[:], op.bitwise_xor)
        y = work.tile([128, C2], u16, tag="y")
        if j % 4 == 0:
            nc.gpsimd.tensor_copy(y[:].bitcast(bf16), x16[:])
        else:
            nc.scalar.copy(y[:].bitcast(bf16), x16[:])
        er = work.tile([128, C2], u16, tag="er")
        nc.vector.tensor_scalar(er[:], y[:], 7, None, op.logical_shift_right)
        t = work.tile([128, C2], f16, tag="t")
        nc.vector.scalar_tensor_tensor(
            t[:], er[:], 142, swc16[:], op.subtract, op.mult
        )
        w = work.tile([128, C2], f16, tag="w")
        for h in range(2):
            jj = h * T + j
            nc.scalar.activation(
                w[:, h * C : (h + 1) * C],
                t[:, h * C : (h + 1) * C],
                AF.Abs,
                bias=btab_sb[:, jj : jj + 1],
                scale=mtab_sb[:, jj : jj + 1],
            )
        for k, (a, bnd) in enumerate(SL):
            nc.tensor.matmul(
                pp[k][:], ident_sb[:], w[:, a:bnd], start=(j == 0), stop=False
            )
    for k, (a, bnd) in enumerate(SL):
        nc.tensor.matmul(
            pp[k][:], ident_sb[:], delta_sb[:, a:bnd], start=False, stop=True
        )
    acc_sb = data.tile([128, C2], f32, tag="accsb")
    for k, (a, bnd) in enumerate(SL):
        nc.scalar.copy(acc_sb[:, a:bnd], pp[k][:])
    nc.sync.dma_start(out[0], acc_sb[:, 0:C])
    nc.sync.dma_start(out[1], acc_sb[:, C:C2])


def _build():
    if "nc" in _compiled:
        return _compiled["nc"]
    from contextlib import ExitStack

    from concourse import bacc, mybir, tile

    nc = bacc.Bacc("TRN2", target_bir_lowering=False, debug=False)
    f32, i32 = mybir.dt.float32, mybir.dt.int32
    u16, f16 = mybir.dt.uint16, mybir.dt.float16
    ins = {
        "cnc": nc.dram_tensor("cnc", [128, 2 * C], i32, kind="ExternalInput").ap(),
        "auf": nc.dram_tensor("auf", [T, 128, 2 * C], u16, kind="ExternalInput").ap(),
        "btab": nc.dram_tensor("btab", [2, 128, T], f32, kind="ExternalInput").ap(),
        "mtab": nc.dram_tensor("mtab", [2, 128, T], f32, kind="ExternalInput").ap(),
        "ident": nc.dram_tensor(
            "ident", [128, 128], f16, kind="ExternalInput"
        ).ap(),
        "delta": nc.dram_tensor(
            "delta", [128, 2 * C], f16, kind="ExternalInput"
        ).ap(),
    }
    outs = {
        "out": nc.dram_tensor("out", [2, 128, C], f32, kind="ExternalOutput").ap()
    }
    with tile.TileContext(nc) as tc:
        with ExitStack() as ctx:
            _emit(ctx, tc, outs, ins)
    nc.compile()
    _compiled["nc"] = nc
    return nc


def _prep_core(b, sta, cnc, logits, lg, mask, lut, exc):
    abs_s = np.abs(sta[b]).astype(np.int64)  # (T, TP)
    su = np.where(sta[b] >= 0, np.float32(1.0), np.float32(-1.0))
    dx = abs_s[:, None, :] ^ abs_s[None, :, :]
    dsp = (su[:, None, :] * su[None, :, :] * (np.float32(1.0) - lut[dx])).astype(
        np.float32
    )  # (i,j,d)
    dsps = dsp.sum(-1, dtype=np.float32)  # (i,j)
    off = (
        (dsps[:, :, None] - dsp) / np.float32(8.0) - logits[b][:, :, None]
    ).astype(np.float32)
    A = (np.float32(128.0) * off * su[None, :, :]).astype(np.float32)  # (i,j,d)
    m = mask[b].astype(np.float32)  # (i,j)
    bt = -(m[:, :, None] * A)  # (i,j,d)
    bt = np.ascontiguousarray(np.transpose(bt, (0, 2, 1))).reshape(2, 128, T)
    mt = np.ascontiguousarray(
        np.broadcast_to(m[:, None, :], (T, TP, T))
    ).reshape(2, 128, T)
    # au[p=(i,d), j] = abs_s[j, d]; replicated over the 16 i-rows of a half
    au_tab = np.ascontiguousarray(
        np.broadcast_to(abs_s.T[None, :, :], (16, TP, T))
    ).reshape(128, T)
    cnc_t = np.ascontiguousarray(np.transpose(cnc[b], (0, 2, 1))).reshape(
        2, 128, C
    )

    # sparse exception fixup, accumulated into PSUM by the delta matmul
    delta = np.zeros((2, 128, C), dtype=np.float64)
    if len(exc):
        aw = np.abs(cnc_t).astype(np.int64)
        swc = np.where(cnc_t >= 0, 1.0, -1.0)
        er3_true = lut.astype(np.float64) * 16.0 - 16.0
        er3_dev = _er3_dev_table().astype(np.float64)
        is_exc = np.zeros(NLUT, dtype=bool)
        is_exc[exc] = True
        # full xor table (h,p,c,j); patch cells hitting exception x-values
        xs = aw[:, :, :, None] ^ au_tab[None, :, None, :].astype(np.int64)
        hh, pp, cc, jj = np.nonzero(is_exc[xs])
        e = xs[hh, pp, cc, jj]
        sc = mt[hh, pp, jj].astype(np.float64)
        bias = bt[hh, pp, jj].astype(np.float64)
        sw = swc[hh, pp, cc]
        dw = np.abs(er3_true[e] * sw * sc + bias) - np.abs(
            er3_dev[e] * sw * sc + bias
        )
        np.add.at(delta, (hh, pp, cc), dw)
    return {
        "cnc": np.ascontiguousarray(
            np.transpose(cnc_t, (1, 0, 2))
        ).reshape(128, 2 * C).astype(np.int32),
        "auf": np.ascontiguousarray(
            np.broadcast_to(au_tab.T[:, :, None], (T, 128, 2 * C))
        ).astype(np.uint16),
        "btab": bt.astype(np.float32),
        "mtab": mt.astype(np.float32),
        "ident": np.eye(128, dtype=np.float16),
        "delta": np.ascontiguousarray(
            np.transpose(delta, (1, 0, 2))
        ).reshape(128, 2 * C).astype(np.float16),
    }


def _numpy_ref(sta, cnc, logits, lg, mask, lut):
    sel = np.zeros(sta.shape[:2] + (sta.shape[2],), dtype=cnc.dtype)
    nb, nt = sta.shape[:2]
    D = sta.shape[-1]
    ml = np.zeros_like(sel, dtype=np.float32)

    def dist(c1, c2):
        sg = np.where((c1 >= 0) == (c2 >= 0), np.float32(1.0), np.float32(-1.0))
        s = lut[np.bitwise_xor(np.abs(c1), np.abs(c2))]
        return (sg * (np.float32(1.0) - s)).astype(np.float32)

    for b in range(nb):
        dsp = dist(sta[b][:, None, :], sta[b][None, :, :])  # (T,T,D)
        dsps = dsp.sum(-1, dtype=np.float32)
        lct = dist(sta[b][None, :, None, :], cnc[b][:, None, :, :])  # (i,j,c,d)
        lct = (lct - dsp[:, :, None, :] + dsps[:, :, None, None]) / np.float32(
            D
        )
        delt = (lct - logits[b][:, :, None, None]) * mask[b][:, :, None, None]
        total = np.abs(delt).sum(axis=1, dtype=np.float32) / lg[b][
            :, None, None
        ].astype(np.float32)  # (i,c,d)
        idx = np.argmin(total, axis=1)  # (i,d)
        sel[b] = np.take_along_axis(cnc[b], idx[:, None, :], axis=1)[:, 0, :]
        ml[b] = np.take_along_axis(total, idx[:, None, :], axis=1)[:, 0, :]
    return sel, ml


def kernel(**inputs):
    sta = np.asarray(inputs["sta_loc"])
    cnc = np.asarray(inputs["cnc_loc"])
    logits = np.asarray(inputs["logits"], dtype=np.float32)
    lg = np.asarray(inputs["lg"])
    mask = np.asarray(inputs["mask"])
    lut = np.asarray(inputs["lut"], dtype=np.float32)

    exc = None
    if lut.shape == (NLUT,):
        exc = _lut_exceptions(lut)
    if (
        exc is None
        or len(exc) > 2048
        or np.any(lg == 0)
        or sta.shape != (B, T, TP)
        or cnc.shape != (B, T, C, TP)
        or np.any(np.abs(sta) >= NLUT)
        or np.any(np.abs(cnc.astype(np.int64)) >= NLUT)
    ):
        return _numpy_ref(sta, cnc, logits, lg, mask, lut)

    from concourse.bass_utils import run_bass_kernel_spmd

    nc = _build()
    in_maps = [
        _prep_core(b, sta, cnc, logits, lg, mask, lut, exc) for b in range(B)
    ]
    res = run_bass_kernel_spmd(nc, in_maps, core_ids=list(range(B))).results
    accs = [np.asarray(res[b]["out"]) for b in range(B)]
    return _finish(accs, sta, cnc, logits, lg, mask, lut)


def _finish(accs, sta, cnc, logits, lg, mask, lut, eps=4.0):
    """Argmin from the device's raw loss accumulator.  Candidates within
    eps (raw units = 128*lg*loss) of the row minimum are recomputed exactly
    reference-style so near-ties resolve identically to the reference."""
    sel = np.zeros((B, T, TP), dtype=cnc.dtype)
    ml = np.zeros((B, T, TP), dtype=np.float32)
    one = np.float32(1.0)
    for b in range(B):
        acc = accs[b].reshape(T, TP, C)  # (i,d,c): (2,16,8,C) flattens to i=(h,16)
        thr = acc.min(axis=-1) + np.float32(eps)
        sub = np.abs(sta[b])  # (T,TP)
        def dist(c1, c2):
            sg = np.where((c1 >= 0) == (c2 >= 0), one, -one)
            s = lut[np.bitwise_xor(np.abs(c1), np.abs(c2))]
            return (sg * (one - s)).astype(np.float32)

        dsp = dist(sta[b][:, None, :], sta[b][None, :, :])  # (i,j,d)
        dsps = dsp.sum(-1, dtype=np.float32)  # (i,j)
        for i in range(T):
            for d in range(TP):
                cs = np.nonzero(acc[i, d] <= thr[i, d])[0]
                q = cnc[b][i, cs, d]  # candidate values
                lct = dist(sta[b][:, d][:, None], q[None, :])  # (j, nc)
                lct = (
                    lct - dsp[i, :, d][:, None] + dsps[i, :][:, None]
                ) / np.float32(TP)
                delt = (lct - logits[b][i][:, None]) * mask[b][i][:, None]
                tot = np.abs(delt).sum(axis=0, dtype=np.float32) / lg[b][
                    i
                ].astype(np.float32)
                k = int(np.argmin(tot))
                sel[b, i, d] = cnc[b][i, cs[k], d]
                ml[b, i, d] = tot[k]
    return sel, ml
